# revision 6
# baseline (speedup 1.0000x reference)
"""Trainium2 Bass kernel for nn_CBAM_86947317940497 (CBAM-style gnn message passing).

Computation (N=100000 points, K=16 knn, C=64 ch, HID=16, 27-nbr sparse conv):
  g = x_F[idx]; gate = sigmoid(mlp(mean_k g) + mlp(max_k g)); outse = x_F*gate
  z = [mean_{k*c} outse[idx], max_{k*c} outse[idx]]
  convf = einsum(z[conv_idx]*mask, conv_w); out = outse * sigmoid(convf)

Distribution: points sharded 8 ways (12500/core, padded to 12544 = 98 tiles
of 128).  x_F replicated.  Three SPMD launches with host concat of the tiny
(N,2) stat tensors between launches:
  L1: knn gather (indirect DMA) -> pool -> MLP gate -> outse + row stats sm.
  L2: gather sm pairs at idx -> z per point.
  L3: points sorted by valid-conv-neighbor count; per-iteration adaptive
      number of z-pair gathers; weight pairs precomputed host-side and
      streamed sequentially.
L1/L2 batch 7 point-tiles per hardware-loop iteration to amortize loop sync
overhead; gathers are SWDGE vector-indirect DMAs (128 rows/call).
"""

from contextlib import ExitStack

import numpy as np

import concourse.bass as bass
import concourse.bacc as bacc
import concourse.mybir as mybir
from concourse.tile import TileContext
from concourse.bass_utils import run_bass_kernel_spmd
from concourse.masks import make_identity

N, K, C, HID = 100000, 16, 64, 16
NCORES = 8
SH = N // NCORES            # 12500 rows per core
P = 128
NT = (SH + P - 1) // P      # 98 tiles
SHP = NT * P                # 12544 padded rows
SMF_ROWS = NCORES * SHP     # 100352
ZF_ROWS = SMF_ROWS
B = 7                       # tiles per loop iteration (98 = 14*7)
NIT = NT // B               # 14 iterations
BP = B * P                  # 896 rows per iteration
EMAXS = 9                   # max valid conv neighbors (self incl.)

F32 = mybir.dt.float32
I32 = mybir.dt.int32


def _nc():
    return bacc.Bacc("TRN2", target_bir_lowering=False, debug=False,
                     num_devices=NCORES)


def build_l1(repeat=1):
    nc = _nc()
    xf = nc.dram_tensor("xf", [N, C], F32, kind="ExternalInput")
    xo = nc.dram_tensor("xo", [SHP, C], F32, kind="ExternalInput")
    ji = nc.dram_tensor("ji", [SHP, K], I32, kind="ExternalInput")
    w1 = nc.dram_tensor("w1", [C, HID], F32, kind="ExternalInput")
    b1 = nc.dram_tensor("b1", [HID, 1], F32, kind="ExternalInput")
    w2 = nc.dram_tensor("w2", [HID, C], F32, kind="ExternalInput")
    b2x2 = nc.dram_tensor("b2x2", [C, 1], F32, kind="ExternalInput")
    outse = nc.dram_tensor("outse", [SHP, C], F32, kind="ExternalOutput")
    sm = nc.dram_tensor("sm", [SHP, 2], F32, kind="ExternalOutput")

    with TileContext(nc) as tc:
        with tc.tile_pool(name="const", bufs=1) as cpool, \
             tc.tile_pool(name="gpool", bufs=2) as gpool, \
             tc.tile_pool(name="sbuf", bufs=3) as pool, \
             tc.tile_pool(name="opool", bufs=2) as opool, \
             tc.tile_pool(name="ipool", bufs=3) as ipool, \
             tc.tile_pool(name="psum", bufs=1, space="PSUM") as ppool:
            idt = cpool.tile([P, P], F32)
            make_identity(nc, idt[:])
            w1s = cpool.tile([C, HID], F32)
            nc.sync.dma_start(out=w1s[:], in_=w1[:])
            b1s = cpool.tile([HID, 1], F32)
            nc.sync.dma_start(out=b1s[:], in_=b1[:])
            w2s = cpool.tile([HID, C], F32)
            nc.sync.dma_start(out=w2s[:], in_=w2[:])
            b2s = cpool.tile([C, 1], F32)
            nc.sync.dma_start(out=b2s[:], in_=b2x2[:])

            rep_ctx = ExitStack()
            if repeat > 1:
                rep_ctx.enter_context(tc.For_i(0, repeat, 1))
            with rep_ctx, tc.For_i(0, NT * P, BP) as r0:
                it = ipool.tile([P, B * K], I32)
                nc.sync.dma_start(
                    out=it[:].rearrange("p (b k) -> p b k", b=B),
                    in_=ji[bass.ds(r0, BP), :].rearrange(
                        "(b p) k -> p b k", p=P))
                gt = gpool.tile([P, B * K * C], F32, tag="g")
                for b in range(B):
                    for j in range(K):
                        q = b * K + j
                        nc.gpsimd.indirect_dma_start(
                            out=gt[:, q * C:(q + 1) * C], out_offset=None,
                            in_=xf[:],
                            in_offset=bass.IndirectOffsetOnAxis(
                                ap=it[:, q:q + 1], axis=0),
                        )
                xt = opool.tile([P, B * C], F32, tag="xt")
                nc.sync.dma_start(
                    out=xt[:].rearrange("p (b c) -> p b c", b=B),
                    in_=xo[bass.ds(r0, BP), :].rearrange(
                        "(b p) c -> p b c", p=P))
                ot = opool.tile([P, B * C], F32, tag="ot")
                smt = opool.tile([P, B * 2], F32, tag="smt")
                for b in range(B):
                    gv = gt[:, b * K * C:(b + 1) * K * C].rearrange(
                        "p (j c) -> p c j", j=K)
                    pm = pool.tile([P, C], F32, tag="pm")
                    nc.vector.tensor_reduce(out=pm[:], in_=gv,
                                            axis=mybir.AxisListType.X,
                                            op=mybir.AluOpType.add)
                    px = pool.tile([P, C], F32, tag="px")
                    nc.vector.tensor_reduce(out=px[:], in_=gv,
                                            axis=mybir.AxisListType.X,
                                            op=mybir.AluOpType.max)
                    ps_m = ppool.tile([C, P], F32, tag="tp1")
                    nc.tensor.transpose(out=ps_m[:], in_=pm[:], identity=idt[:])
                    ps_x = ppool.tile([C, P], F32, tag="tp2")
                    nc.tensor.transpose(out=ps_x[:], in_=px[:], identity=idt[:])
                    poolT = pool.tile([C, 2 * P], F32, tag="poolT")
                    nc.scalar.activation(out=poolT[:, 0:P], in_=ps_m[:],
                                         func=mybir.ActivationFunctionType.Copy,
                                         scale=1.0 / K)
                    nc.vector.tensor_copy(out=poolT[:, P:2 * P], in_=ps_x[:])
                    ps1 = ppool.tile([HID, 2 * P], F32, tag="mm1")
                    nc.tensor.matmul(out=ps1[:], lhsT=w1s[:], rhs=poolT[:],
                                     start=True, stop=True)
                    h = pool.tile([HID, 2 * P], F32, tag="h")
                    nc.scalar.activation(out=h[:], in_=ps1[:],
                                         func=mybir.ActivationFunctionType.Relu,
                                         bias=b1s[:])
                    ps2 = ppool.tile([C, 2 * P], F32, tag="mm2")
                    nc.tensor.matmul(out=ps2[:], lhsT=w2s[:], rhs=h[:],
                                     start=True, stop=True)
                    g2 = pool.tile([C, 2 * P], F32, tag="g2")
                    nc.vector.tensor_copy(out=g2[:], in_=ps2[:])
                    pre = pool.tile([C, P], F32, tag="pre")
                    nc.vector.tensor_add(out=pre[:], in0=g2[:, 0:P],
                                         in1=g2[:, P:2 * P])
                    gT = pool.tile([C, P], F32, tag="gT")
                    nc.scalar.activation(out=gT[:], in_=pre[:],
                                         func=mybir.ActivationFunctionType.Sigmoid,
                                         bias=b2s[:])
                    psg = ppool.tile([P, C], F32, tag="tpg")
                    nc.tensor.transpose(out=psg[:], in_=gT[:],
                                        identity=idt[0:C, 0:C])
                    gate = pool.tile([P, C], F32, tag="gate")
                    nc.vector.tensor_copy(out=gate[:], in_=psg[:])
                    nc.vector.tensor_mul(out=ot[:, b * C:(b + 1) * C],
                                         in0=xt[:, b * C:(b + 1) * C],
                                         in1=gate[:])
                    s0 = pool.tile([P, 1], F32, tag="s0")
                    nc.vector.tensor_reduce(out=s0[:],
                                            in_=ot[:, b * C:(b + 1) * C],
                                            axis=mybir.AxisListType.X,
                                            op=mybir.AluOpType.add)
                    nc.scalar.activation(out=smt[:, 2 * b:2 * b + 1],
                                         in_=s0[:],
                                         func=mybir.ActivationFunctionType.Copy,
                                         scale=1.0 / C)
                    nc.vector.tensor_reduce(out=smt[:, 2 * b + 1:2 * b + 2],
                                            in_=ot[:, b * C:(b + 1) * C],
                                            axis=mybir.AxisListType.X,
                                            op=mybir.AluOpType.max)
                nc.sync.dma_start(
                    out=outse[bass.ds(r0, BP), :].rearrange(
                        "(b p) c -> p b c", p=P),
                    in_=ot[:].rearrange("p (b c) -> p b c", b=B))
                nc.sync.dma_start(
                    out=sm[bass.ds(r0, BP), :].rearrange(
                        "(b p) c -> p b c", p=P),
                    in_=smt[:].rearrange("p (b c) -> p b c", b=B))
    nc.compile()
    return nc


def build_l2(repeat=1):
    nc = _nc()
    smf = nc.dram_tensor("smf", [SMF_ROWS, 2], F32, kind="ExternalInput")
    ji2 = nc.dram_tensor("ji2", [SHP, K], I32, kind="ExternalInput")
    z = nc.dram_tensor("z", [SHP, 2], F32, kind="ExternalOutput")
    with TileContext(nc) as tc:
        with tc.tile_pool(name="sbuf", bufs=3) as pool, \
             tc.tile_pool(name="gpool", bufs=3) as gpool, \
             tc.tile_pool(name="ipool", bufs=3) as ipool:
            rep_ctx = ExitStack()
            if repeat > 1:
                rep_ctx.enter_context(tc.For_i(0, repeat, 1))
            with rep_ctx, tc.For_i(0, NT * P, BP) as r0:
                it = ipool.tile([P, B * K], I32)
                nc.sync.dma_start(
                    out=it[:].rearrange("p (b k) -> p b k", b=B),
                    in_=ji2[bass.ds(r0, BP), :].rearrange(
                        "(b p) k -> p b k", p=P))
                sg = gpool.tile([P, B * K * 2], F32, tag="sg")
                for q in range(B * K):
                    nc.gpsimd.indirect_dma_start(
                        out=sg[:, q * 2:(q + 1) * 2], out_offset=None,
                        in_=smf[:],
                        in_offset=bass.IndirectOffsetOnAxis(
                            ap=it[:, q:q + 1], axis=0),
                    )
                zt = pool.tile([P, B * 2], F32, tag="zt")
                for b in range(B):
                    sv = sg[:, b * K * 2:(b + 1) * K * 2].rearrange(
                        "p (j c) -> p c j", j=K)
                    rs = pool.tile([P, 2], F32, tag="rs")
                    nc.vector.tensor_reduce(out=rs[:], in_=sv,
                                            axis=mybir.AxisListType.X,
                                            op=mybir.AluOpType.add)
                    rm = pool.tile([P, 2], F32, tag="rm")
                    nc.vector.tensor_reduce(out=rm[:], in_=sv,
                                            axis=mybir.AxisListType.X,
                                            op=mybir.AluOpType.max)
                    nc.scalar.activation(out=zt[:, 2 * b:2 * b + 1],
                                         in_=rs[:, 0:1],
                                         func=mybir.ActivationFunctionType.Copy,
                                         scale=1.0 / K)
                    nc.vector.tensor_copy(out=zt[:, 2 * b + 1:2 * b + 2],
                                          in_=rm[:, 1:2])
                nc.sync.dma_start(
                    out=z[bass.ds(r0, BP), :].rearrange(
                        "(b p) c -> p b c", p=P),
                    in_=zt[:].rearrange("p (b c) -> p b c", b=B))
    nc.compile()
    return nc


def build_l3(emaxs, repeat=1):
    """emaxs: list of NIT per-iteration gather counts (max valid count incl
    self over the iteration's 896 sorted points)."""
    nc = _nc()
    zf = nc.dram_tensor("zf", [ZF_ROWS, 2], F32, kind="ExternalInput")
    ei = nc.dram_tensor("ei", [SHP, EMAXS], I32, kind="ExternalInput")
    wnt = nc.dram_tensor("wnt", [SHP, 2 * EMAXS], F32, kind="ExternalInput")
    oi = nc.dram_tensor("oi", [SHP, C], F32, kind="ExternalInput")
    out = nc.dram_tensor("out", [SHP, C], F32, kind="ExternalOutput")

    with TileContext(nc) as tc:
        with tc.tile_pool(name="sbuf", bufs=3) as pool, \
             tc.tile_pool(name="gpool", bufs=3) as gpool, \
             tc.tile_pool(name="opool", bufs=2) as opool, \
             tc.tile_pool(name="ipool", bufs=3) as ipool:
            rep_ctx = ExitStack()
            if repeat > 1:
                rep_ctx.enter_context(tc.For_i(0, repeat, 1))
            with rep_ctx:
                for i_it in range(NIT):
                    r0 = i_it * BP
                    em = max(int(emaxs[i_it]), 1)
                    et = ipool.tile([P, B * em], I32, tag="et")
                    nc.sync.dma_start(
                        out=et[:].rearrange("p (b e) -> p b e", b=B),
                        in_=ei[r0:r0 + BP, 0:em].rearrange(
                            "(b p) e -> p b e", p=P))
                    wn = pool.tile([P, B * 2 * em], F32, tag="wn")
                    nc.sync.dma_start(
                        out=wn[:].rearrange("p (b e) -> p b e", b=B),
                        in_=wnt[r0:r0 + BP, 0:2 * em].rearrange(
                            "(b p) e -> p b e", p=P))
                    zn = gpool.tile([P, B * em * 2], F32, tag="zn")
                    for b in range(B):
                        for e in range(em):
                            q = b * em + e
                            nc.gpsimd.indirect_dma_start(
                                out=zn[:, q * 2:(q + 1) * 2], out_offset=None,
                                in_=zf[:],
                                in_offset=bass.IndirectOffsetOnAxis(
                                    ap=et[:, q:q + 1], axis=0),
                            )
                    ot = opool.tile([P, B * C], F32, tag="ot")
                    nc.sync.dma_start(
                        out=ot[:].rearrange("p (b c) -> p b c", b=B),
                        in_=oi[r0:r0 + BP, :].rearrange(
                            "(b p) c -> p b c", p=P))
                    pr = pool.tile([P, B * em * 2], F32, tag="pr")
                    nc.vector.tensor_mul(out=pr[:], in0=zn[:], in1=wn[:])
                    ft = opool.tile([P, B * C], F32, tag="ft")
                    for b in range(B):
                        cf = pool.tile([P, 1], F32, tag="cf")
                        nc.vector.tensor_reduce(
                            out=cf[:], in_=pr[:, b * em * 2:(b + 1) * em * 2],
                            axis=mybir.AxisListType.X, op=mybir.AluOpType.add)
                        sg_ = pool.tile([P, 1], F32, tag="sig")
                        nc.scalar.activation(
                            out=sg_[:], in_=cf[:],
                            func=mybir.ActivationFunctionType.Sigmoid)
                        nc.vector.tensor_mul(
                            out=ft[:, b * C:(b + 1) * C],
                            in0=ot[:, b * C:(b + 1) * C],
                            in1=sg_[:].to_broadcast([P, C]))
                    nc.sync.dma_start(
                        out=out[r0:r0 + BP, :].rearrange(
                            "(b p) c -> p b c", p=P),
                        in_=ft[:].rearrange("p (b c) -> p b c", b=B))
    nc.compile()
    return nc


def _pad_rows(a, rows):
    out = np.zeros((rows,) + a.shape[1:], a.dtype)
    out[:a.shape[0]] = a
    return out


def kernel(x_F, W1, b1, W2, b2, conv_w, idx, conv_idx):
    x_F = np.ascontiguousarray(np.asarray(x_F, dtype=np.float32))
    W1 = np.asarray(W1, dtype=np.float32)
    b1 = np.asarray(b1, dtype=np.float32)
    W2 = np.asarray(W2, dtype=np.float32)
    b2 = np.asarray(b2, dtype=np.float32)
    conv_w = np.asarray(conv_w, dtype=np.float32).reshape(27, 2)
    idx = np.asarray(idx).astype(np.int32)
    conv_idx = np.asarray(conv_idx).astype(np.int32)

    cores = list(range(NCORES))
    qmap = lambda n: (n // SH) * SHP + (n % SH)          # noqa: E731
    idx_q = qmap(idx.astype(np.int64)).astype(np.int32)
    valid = conv_idx >= 0
    ci_q = np.where(valid, qmap(np.clip(conv_idx, 0, None).astype(np.int64)),
                    0).astype(np.int32)
    # compact valid conv entries to the first slots per row
    order = np.argsort(~valid, axis=1, kind="stable")
    eiz = np.take_along_axis(ci_q, order, axis=1)[:, :EMAXS]
    lro = np.take_along_axis(
        np.where(valid,
                 np.broadcast_to(np.arange(27, dtype=np.int32),
                                 conv_idx.shape), 27).astype(np.int32),
        order, axis=1)[:, :EMAXS]
    cw28 = np.zeros((28, 2), np.float32)
    cw28[:27] = conv_w
    wvals = cw28[lro]                       # (N, EMAXS, 2), zeros on pad
    wnt_full = np.ascontiguousarray(
        wvals.reshape(N, 2 * EMAXS))
    cnt = valid.sum(1)

    # ---- L1
    nc1 = build_l1()
    in1 = []
    for c in cores:
        sl = slice(c * SH, (c + 1) * SH)
        in1.append({
            "xf": x_F,
            "xo": _pad_rows(x_F[sl], SHP),
            "ji": _pad_rows(idx[sl], SHP),
            "w1": W1,
            "b1": b1.reshape(HID, 1),
            "w2": W2,
            "b2x2": (2.0 * b2).reshape(C, 1),
        })
    r1 = run_bass_kernel_spmd(nc1, in1, core_ids=cores)
    outse = np.concatenate([r1.results[c]["outse"] for c in cores], 0)
    smf = np.concatenate([r1.results[c]["sm"] for c in cores], 0)

    # ---- L2
    nc2 = build_l2()
    in2 = []
    for c in cores:
        sl = slice(c * SH, (c + 1) * SH)
        in2.append({"smf": smf, "ji2": _pad_rows(idx_q[sl], SHP)})
    r2 = run_bass_kernel_spmd(nc2, in2, core_ids=cores)
    zf = np.zeros((ZF_ROWS, 2), np.float32)
    for c in cores:
        zc = r2.results[c]["z"]
        zf[c * SHP: c * SHP + SH] = zc[:SH]   # zero the pad rows

    # ---- L3: per-core sort by valid count (desc), adaptive gather counts
    perms = []
    emaxs_all = np.zeros(NIT, np.int64)
    in3 = []
    for c in cores:
        sl = slice(c * SH, (c + 1) * SH)
        cnt_c = _pad_rows(cnt[sl].astype(np.int32), SHP)
        perm = np.argsort(-cnt_c, kind="stable")
        perms.append(perm)
        cnt_s = cnt_c[perm]
        for i_it in range(NIT):
            m = int(cnt_s[i_it * BP: (i_it + 1) * BP].max())
            emaxs_all[i_it] = max(emaxs_all[i_it], m)
        in3.append({
            "zf": zf,
            "ei": _pad_rows(eiz[sl], SHP)[perm],
            "wnt": _pad_rows(wnt_full[sl], SHP)[perm],
            "oi": outse[c * SHP:(c + 1) * SHP][perm],
        })
    nc3 = build_l3(emaxs=emaxs_all)
    r3 = run_bass_kernel_spmd(nc3, in3, core_ids=cores)
    out = np.empty((N, C), np.float32)
    for c in cores:
        o_p = r3.results[c]["out"]
        o = np.empty_like(o_p)
        o[perms[c]] = o_p
        out[c * SH:(c + 1) * SH] = o[:SH]
    return out
